# revision 12
# baseline (speedup 1.0000x reference)
"""ARD-RBF kernel matrix on 8 Trainium2 NeuronCores.

out = variance * exp(-0.5 * (sq1[:,None] + sq2[None,:] - 2*cross))
    = exp(cross + (-0.5*sq1 + ln(var))[:,None] + (-0.5*sq2)[None,:])

with alpha = softmax(softplus(alpha_raw)), variance = variance_raw[0]**2,
cross = (x1*alpha) @ x2.T, sq1 = (x1*x1)@alpha, sq2 = (x2*x2)@alpha.

Device strategy (per core, rows of x1 sharded 8 ways):
  - host ships x1.T shard [512, 1024] f32 and x2.T [512, 8192] bf16
    (pure layout/precision prep; every FLOP of the reference runs on
    device).
  - alpha/softplus/softmax computed on-chip on a [1,512] row; scattered
    to [128,4] per-partition layout via a DRAM bounce.
  - r1 = -0.5*sq1 + ln(var) via M=1 matmuls (alpha as stationary) plus a
    K=2 hi/lo matmul carrying ln(var); applied as the per-partition f32
    bias of the final ScalarE Exp activation.
  - c2 = -0.5*sq2 via M=1 matmuls; split hi/lo into two bf16 rows that
    augment the main GEMM's contraction (exactly adds c2[j] per column).
  - main GEMM: for each output tile, 4 K-chunk bf16 matmuls + 1 K=2
    augment matmul accumulate in PSUM; one Exp activation PSUM->SBUF with
    bias; DMA to DRAM.
"""

import os
import sys

import numpy as np

sys.path.insert(0, "/opt/trn_rl_repo")

import ml_dtypes

N_CORES = 8
N_ROWS, M_COLS, DIM = 8192, 8192, 512
ROWS = N_ROWS // N_CORES  # 1024 rows of x1 per core


def build_ard_rbf(tc, out, x1t, x2t, araw, vraw, rows, m_cols, dim):
    """Emit the per-core kernel. APs: out [rows, m_cols] f32,
    x1t [dim, rows] f32, x2t [dim, m_cols] bf16, araw [dim] f32,
    vraw [1] f32."""
    import concourse.mybir as mybir

    nc = tc.nc
    f32 = mybir.dt.float32
    bf16 = mybir.dt.bfloat16
    AF = mybir.ActivationFunctionType
    AX = mybir.AxisListType

    KC = dim // 128          # contraction chunks (4)
    MT = rows // 128         # output row tiles per core (8)
    NG = m_cols // 1024      # psum groups of 1024 columns (8)

    with (
        tc.tile_pool(name="const", bufs=1) as const,
        tc.tile_pool(name="x2pool", bufs=1) as x2pool,
        tc.tile_pool(name="work", bufs=3) as work,
        tc.tile_pool(name="outp", bufs=4) as outp,
        tc.tile_pool(name="psum", bufs=3, space="PSUM") as psum,
        tc.tile_pool(name="psmall", bufs=2, space="PSUM") as psmall,
        tc.tile_pool(name="dramp", bufs=1, space="DRAM") as dram,
    ):
        # ---------------- input loads first (no deps; keeps DMA queues
        # busy from t=0 instead of head-of-line blocking behind the alpha
        # chain's small dependent DMAs) ----------------------------------
        x2_c = [
            x2pool.tile([128, m_cols], bf16, tag=f"x2{k}", name=f"x2_{k}")
            for k in range(KC)
        ]
        for g in range(NG):
            gsl = slice(g * 1024, (g + 1) * 1024)
            for k in range(KC):
                nc.sync.dma_start(
                    out=x2_c[k][:, gsl],
                    in_=x2t[k * 128 : (k + 1) * 128, gsl],
                )
        x1t_c = []
        for k in range(KC):
            xt = const.tile([128, rows], f32, tag=f"x1t{k}", name=f"x1t_{k}")
            nc.sync.dma_start(out=xt, in_=x1t[k * 128 : (k + 1) * 128, :])
            x1t_c.append(xt)

        # ---------------- alpha = softmax(softplus(alpha_raw)) -------------
        # softmax(softplus(x)) = (1+e^x) / sum(1+e^x): exp once, no ln/max
        # needed (|x| <= ~5 here so e^x can't overflow f32).
        a_row = const.tile([1, dim], f32)
        nc.scalar.dma_start(out=a_row, in_=araw.rearrange("(a d) -> a d", a=1))
        e0 = const.tile([1, dim], f32)
        nc.scalar.activation(e0, a_row, AF.Exp)
        sm = const.tile([1, 1], f32)
        nc.vector.reduce_sum(out=sm, in_=e0, axis=AX.X)
        smd = const.tile([1, 1], f32)
        nc.vector.tensor_scalar_add(smd, sm, float(dim))
        rs = const.tile([1, 1], f32)
        nc.vector.reciprocal(rs, smd)
        alpha_row = const.tile([1, dim], f32)
        nc.vector.tensor_scalar(
            alpha_row, e0, rs, rs,
            op0=mybir.AluOpType.mult, op1=mybir.AluOpType.add,
        )

        # scatter [1, dim] -> [128, KC] (partition-major) via DRAM bounce
        abounce = dram.tile([dim], f32)
        nc.scalar.dma_start(out=abounce, in_=alpha_row)
        alpha_p = const.tile([128, KC], f32)
        nc.scalar.dma_start(out=alpha_p, in_=abounce.rearrange("(k p) -> p k", p=128))
        aneg_p = const.tile([128, KC], bf16)
        nc.vector.tensor_scalar_mul(aneg_p, alpha_p, -0.5)

        # ---------------- ln(variance), split hi/lo ------------------------
        vr = const.tile([1, 1], f32)
        nc.scalar.dma_start(out=vr, in_=vraw.rearrange("(a d) -> a d", a=1))
        var = const.tile([1, 1], f32)
        nc.vector.tensor_mul(var, vr, vr)
        lnv = const.tile([1, 1], f32)
        nc.scalar.activation(lnv, var, AF.Ln)
        lv_hi_b = const.tile([1, 1], bf16)
        nc.vector.tensor_copy(lv_hi_b, lnv)
        lv_hi_f = const.tile([1, 1], f32)
        nc.vector.tensor_copy(lv_hi_f, lv_hi_b)
        lv_lo_b = const.tile([1, 1], bf16)
        nc.vector.tensor_sub(lv_lo_b, lnv, lv_hi_f)
        lv_hilo = const.tile([2, 1], bf16)
        nc.scalar.dma_start(out=lv_hilo[0:1, :], in_=lv_hi_b)
        nc.scalar.dma_start(out=lv_hilo[1:2, :], in_=lv_lo_b)

        ones2 = const.tile([2, 512], bf16)
        nc.vector.memset(ones2, 1.0)

        # ---------------- x1 chunks: scale to bf16 -------------------------
        x1a_c = []
        for k in range(KC):
            xa = const.tile([128, rows], bf16, tag=f"x1a{k}", name=f"x1a_{k}")
            nc.vector.tensor_scalar_mul(xa, x1t_c[k], alpha_p[:, k : k + 1])
            x1a_c.append(xa)

        # ---------------- r1 = -0.5*sq1 + ln(var), scatter to [128, MT] ----
        r1_row = const.tile([1, rows], f32)
        for h in range(rows // 512):
            ps = psmall.tile([1, 512], f32, tag="c2ps")
            for k in range(KC):
                sq = work.tile([128, 512], bf16, tag="sqx1")
                src = x1t_c[k][:, h * 512 : (h + 1) * 512]
                nc.vector.tensor_mul(sq, src, src)
                nc.tensor.matmul(
                    ps, lhsT=aneg_p[:, k : k + 1], rhs=sq,
                    start=(k == 0), stop=False,
                )
            nc.tensor.matmul(ps, lhsT=lv_hilo, rhs=ones2, start=False, stop=True)
            nc.vector.tensor_copy(r1_row[:, h * 512 : (h + 1) * 512], ps)
        r1b = dram.tile([rows], f32)
        nc.scalar.dma_start(out=r1b, in_=r1_row)
        r1_t = const.tile([128, MT], f32)
        nc.scalar.dma_start(out=r1_t, in_=r1b.rearrange("(t p) -> p t", p=128))

        # broadcast copy of exp(c2) (whole-row, written per super-group)
        ec2b = const.tile([128, m_cols], f32)

        # ---------------- main loop: 2048-col super-groups outer, m inner --
        NS = m_cols // 2048
        for s2 in range(NS):
            ssl = slice(s2 * 2048, (s2 + 1) * 2048)
            ec2_row = work.tile([1, 2048], f32, tag="ec2row", bufs=2)
            for g in range(2):
                g0 = s2 * 2048 + g * 1024
                sq2g = []
                for k in range(KC):
                    sq2 = work.tile(
                        [128, 1024], bf16, tag=f"sqx2{k}", bufs=2,
                        name=f"sq2_{k}",
                    )
                    nc.vector.tensor_mul(
                        sq2, x2_c[k][:, g0 : g0 + 1024], x2_c[k][:, g0 : g0 + 1024]
                    )
                    sq2g.append(sq2)
                for h in range(2):
                    cps = psmall.tile([1, 512], f32, tag="c2ps")
                    hs = slice(h * 512, (h + 1) * 512)
                    for k in range(KC):
                        nc.tensor.matmul(
                            cps, lhsT=aneg_p[:, k : k + 1], rhs=sq2g[k][:, hs],
                            start=(k == 0), stop=(k == KC - 1),
                        )
                    nc.scalar.activation(
                        ec2_row[:, g * 1024 + h * 512 : g * 1024 + (h + 1) * 512],
                        cps, AF.Exp,
                    )
            nc.gpsimd.partition_broadcast(ec2b[:, ssl], ec2_row)

            last = s2 == NS - 1
            for m in range(MT):
                ot = None if last else outp.tile([128, 2048], f32, tag="ot",
                                                 bufs=3, name="ot")
                for g in range(2):
                    g0 = s2 * 2048 + g * 1024
                    ps = psum.tile([128, 1024], f32, tag="mainps")
                    for h in range(2):
                        sl = slice(g0 + h * 512, g0 + (h + 1) * 512)
                        pslice = ps[:, h * 512 : (h + 1) * 512]
                        for k in range(KC):
                            nc.tensor.matmul(
                                pslice,
                                lhsT=x1a_c[k][:, m * 128 : (m + 1) * 128],
                                rhs=x2_c[k][:, sl],
                                start=(k == 0), stop=(k == KC - 1),
                            )
                    if last:
                        # shorter per-tile drain chain for the final group
                        oth = outp.tile([128, 1024], f32, tag="oth", bufs=3,
                                        name="oth")
                        nc.scalar.activation(oth, ps, AF.Exp,
                                             bias=r1_t[:, m : m + 1])
                        nc.vector.tensor_mul(
                            oth, oth, ec2b[:, g0 : g0 + 1024])
                        nc.sync.dma_start(
                            out=out[m * 128 : (m + 1) * 128, g0 : g0 + 1024],
                            in_=oth)
                    else:
                        nc.scalar.activation(
                            ot[:, g * 1024 : (g + 1) * 1024], ps, AF.Exp,
                            bias=r1_t[:, m : m + 1],
                        )
                if not last:
                    nc.vector.tensor_mul(ot, ot, ec2b[:, ssl])
                    nc.sync.dma_start(
                        out=out[m * 128 : (m + 1) * 128, ssl], in_=ot)


_CACHE = {}


def _get_compiled():
    if "nc" in _CACHE:
        return _CACHE["nc"]
    import concourse.mybir as mybir
    import concourse.tile as tile
    from concourse import bacc

    f32 = mybir.dt.float32
    bf16 = mybir.dt.bfloat16
    nc = bacc.Bacc("TRN2", target_bir_lowering=False, debug=False,
                   enable_asserts=False)
    x1t = nc.dram_tensor("x1t", [DIM, ROWS], f32, kind="ExternalInput").ap()
    x2t = nc.dram_tensor("x2t", [DIM, M_COLS], bf16, kind="ExternalInput").ap()
    araw = nc.dram_tensor("alpha_raw", [DIM], f32, kind="ExternalInput").ap()
    vraw = nc.dram_tensor("variance_raw", [1], f32, kind="ExternalInput").ap()
    out = nc.dram_tensor("out", [ROWS, M_COLS], f32, kind="ExternalOutput").ap()

    with tile.TileContext(nc) as tc:
        build_ard_rbf(tc, out, x1t, x2t, araw, vraw, ROWS, M_COLS, DIM)
    nc.compile()
    _CACHE["nc"] = nc
    return nc


def kernel(x1, x2, alpha_raw, variance_raw):
    from concourse import bass_utils

    x1 = np.asarray(x1, dtype=np.float32)
    x2 = np.asarray(x2, dtype=np.float32)
    alpha_raw = np.ascontiguousarray(np.asarray(alpha_raw, dtype=np.float32))
    variance_raw = np.ascontiguousarray(
        np.asarray(variance_raw, dtype=np.float32))

    x1t_full = np.ascontiguousarray(x1.T)                      # [512, 8192] f32
    x2t_full = np.ascontiguousarray(x2.T).astype(ml_dtypes.bfloat16)

    nc = _get_compiled()
    in_maps = []
    for c in range(N_CORES):
        in_maps.append({
            "x1t": np.ascontiguousarray(x1t_full[:, c * ROWS : (c + 1) * ROWS]),
            "x2t": x2t_full,
            "alpha_raw": alpha_raw,
            "variance_raw": variance_raw,
        })
    res = bass_utils.run_bass_kernel_spmd(
        nc, in_maps, core_ids=list(range(N_CORES)),
        trace=bool(int(os.environ.get("ARD_TRACE", "0"))),
        tmpdir=os.environ.get("ARD_TMPDIR"),
    )
    _CACHE["last_results"] = res
    out = np.concatenate([res.results[c]["out"] for c in range(N_CORES)], axis=0)
    return out


if __name__ == "__main__":
    rng = np.random.default_rng(0)
    ins = {
        "x1": rng.standard_normal((N_ROWS, DIM), dtype=np.float32),
        "x2": rng.standard_normal((M_COLS, DIM), dtype=np.float32),
        "alpha_raw": rng.standard_normal((DIM,), dtype=np.float32),
        "variance_raw": rng.random((1,), dtype=np.float32),
    }
    o = kernel(**ins)
    print(o.shape, o.dtype)


# revision 14
# speedup vs baseline: 1.0217x; 1.0217x over previous
"""ARD-RBF kernel matrix on 8 Trainium2 NeuronCores.

out = variance * exp(-0.5 * (sq1[:,None] + sq2[None,:] - 2*cross))
    = exp(cross + (-0.5*sq1 + ln(var))[:,None] + (-0.5*sq2)[None,:])

with alpha = softmax(softplus(alpha_raw)), variance = variance_raw[0]**2,
cross = (x1*alpha) @ x2.T, sq1 = (x1*x1)@alpha, sq2 = (x2*x2)@alpha.

Device strategy (per core, rows of x1 sharded 8 ways):
  - host ships x1.T shard [512, 1024] f32 and x2.T [512, 8192] bf16
    (pure layout/precision prep; every FLOP of the reference runs on
    device).
  - alpha/softplus/softmax computed on-chip on a [1,512] row; scattered
    to [128,4] per-partition layout via a DRAM bounce.
  - r1 = -0.5*sq1 + ln(var) via M=1 matmuls (alpha as stationary) plus a
    K=2 hi/lo matmul carrying ln(var); applied as the per-partition f32
    bias of the final ScalarE Exp activation.
  - c2 = -0.5*sq2 via M=1 matmuls; split hi/lo into two bf16 rows that
    augment the main GEMM's contraction (exactly adds c2[j] per column).
  - main GEMM: for each output tile, 4 K-chunk bf16 matmuls + 1 K=2
    augment matmul accumulate in PSUM; one Exp activation PSUM->SBUF with
    bias; DMA to DRAM.
"""

import os
import sys

import numpy as np

sys.path.insert(0, "/opt/trn_rl_repo")

import ml_dtypes

N_CORES = 8
N_ROWS, M_COLS, DIM = 8192, 8192, 512
ROWS = N_ROWS // N_CORES  # 1024 rows of x1 per core


def build_ard_rbf(tc, out, x1t, x2t, araw, vraw, rows, m_cols, dim):
    """Emit the per-core kernel. APs: out [rows, m_cols] f32,
    x1t [dim, rows] f32, x2t [dim, m_cols] bf16, araw [dim] f32,
    vraw [1] f32."""
    import concourse.mybir as mybir

    nc = tc.nc
    f32 = mybir.dt.float32
    bf16 = mybir.dt.bfloat16
    AF = mybir.ActivationFunctionType
    AX = mybir.AxisListType

    KC = dim // 128          # contraction chunks (4)
    MT = rows // 128         # output row tiles per core (8)
    NG = m_cols // 1024      # psum groups of 1024 columns (8)

    with (
        tc.tile_pool(name="const", bufs=1) as const,
        tc.tile_pool(name="x2pool", bufs=1) as x2pool,
        tc.tile_pool(name="work", bufs=3) as work,
        tc.tile_pool(name="outp", bufs=4) as outp,
        tc.tile_pool(name="psum", bufs=3, space="PSUM") as psum,
        tc.tile_pool(name="psmall", bufs=2, space="PSUM") as psmall,
        tc.tile_pool(name="dramp", bufs=1, space="DRAM") as dram,
    ):
        # ---------------- input loads first (no deps; keeps DMA queues
        # busy from t=0 instead of head-of-line blocking behind the alpha
        # chain's small dependent DMAs) ----------------------------------
        x2_c = [
            x2pool.tile([128, m_cols], bf16, tag=f"x2{k}", name=f"x2_{k}")
            for k in range(KC)
        ]
        for g in range(NG):
            gsl = slice(g * 1024, (g + 1) * 1024)
            for k in range(KC):
                nc.sync.dma_start(
                    out=x2_c[k][:, gsl],
                    in_=x2t[k * 128 : (k + 1) * 128, gsl],
                )
        x1t_c = []
        for k in range(KC):
            xt = const.tile([128, rows], f32, tag=f"x1t{k}", name=f"x1t_{k}")
            nc.sync.dma_start(out=xt, in_=x1t[k * 128 : (k + 1) * 128, :])
            x1t_c.append(xt)

        # ---------------- alpha = softmax(softplus(alpha_raw)) -------------
        # softmax(softplus(x)) = (1+e^x) / sum(1+e^x): exp once, no ln/max
        # needed (|x| <= ~5 here so e^x can't overflow f32).
        a_row = const.tile([1, dim], f32)
        nc.gpsimd.dma_start(out=a_row, in_=araw.rearrange("(a d) -> a d", a=1))
        e0 = const.tile([1, dim], f32)
        nc.scalar.activation(e0, a_row, AF.Exp)
        sm = const.tile([1, 1], f32)
        nc.vector.reduce_sum(out=sm, in_=e0, axis=AX.X)
        smd = const.tile([1, 1], f32)
        nc.vector.tensor_scalar_add(smd, sm, float(dim))
        rs = const.tile([1, 1], f32)
        nc.vector.reciprocal(rs, smd)
        alpha_row = const.tile([1, dim], f32)
        nc.vector.tensor_scalar(
            alpha_row, e0, rs, rs,
            op0=mybir.AluOpType.mult, op1=mybir.AluOpType.add,
        )

        # scatter [1, dim] -> [128, KC] (partition-major) via a DRAM bounce
        # on the SWDGE path (own queues/sems -> not FIFO'd behind the bulk
        # x2 loads on the HWDGE channels)
        abounce = dram.tile([dim], f32)
        nc.gpsimd.dma_start(out=abounce, in_=alpha_row)
        alpha_p = const.tile([128, KC], f32)
        nc.gpsimd.dma_start(
            out=alpha_p, in_=abounce.rearrange("(k p) -> p k", p=128))
        aneg_p = const.tile([128, KC], bf16)
        nc.vector.tensor_scalar_mul(aneg_p, alpha_p, -0.5)

        # variance multiplies exp(c2) later (keeps Ln off the act-table
        # critical path entirely)
        vr = const.tile([1, 1], f32)
        nc.gpsimd.dma_start(out=vr, in_=vraw.rearrange("(a d) -> a d", a=1))
        var = const.tile([1, 1], f32)
        nc.vector.tensor_mul(var, vr, vr)

        # ---------------- x1 chunks: scale to bf16 -------------------------
        x1a_c = []
        for k in range(KC):
            xa = const.tile([128, rows], bf16, tag=f"x1a{k}", name=f"x1a_{k}")
            nc.vector.tensor_scalar_mul(xa, x1t_c[k], alpha_p[:, k : k + 1])
            x1a_c.append(xa)

        # ---------------- r1 = -0.5*sq1 + ln(var), scatter to [128, MT] ----
        r1_row = const.tile([1, rows], f32)
        for h in range(rows // 512):
            ps = psmall.tile([1, 512], f32, tag="c2ps")
            for k in range(KC):
                sq = work.tile([128, 512], bf16, tag="sqx1")
                src = x1t_c[k][:, h * 512 : (h + 1) * 512]
                nc.vector.tensor_mul(sq, src, src)
                nc.tensor.matmul(
                    ps, lhsT=aneg_p[:, k : k + 1], rhs=sq,
                    start=(k == 0), stop=(k == KC - 1),
                )
            nc.vector.tensor_copy(r1_row[:, h * 512 : (h + 1) * 512], ps)
        r1b = dram.tile([rows], f32)
        nc.gpsimd.dma_start(out=r1b, in_=r1_row)
        r1_t = const.tile([128, MT], f32)
        nc.gpsimd.dma_start(out=r1_t, in_=r1b.rearrange("(t p) -> p t", p=128))

        # broadcast copy of exp(c2) (whole-row, written per super-group)
        ec2b = const.tile([128, m_cols], f32)

        # ---------------- main loop: 2048-col super-groups outer, m inner --
        NS = m_cols // 2048
        for s2 in range(NS):
            ssl = slice(s2 * 2048, (s2 + 1) * 2048)
            ec2_row = work.tile([1, 2048], f32, tag="ec2row", bufs=2)
            for g in range(2):
                g0 = s2 * 2048 + g * 1024
                sq2g = []
                for k in range(KC):
                    sq2 = work.tile(
                        [128, 1024], bf16, tag=f"sqx2{k}", bufs=2,
                        name=f"sq2_{k}",
                    )
                    nc.vector.tensor_mul(
                        sq2, x2_c[k][:, g0 : g0 + 1024], x2_c[k][:, g0 : g0 + 1024]
                    )
                    sq2g.append(sq2)
                for h in range(2):
                    cps = psmall.tile([1, 512], f32, tag="c2ps")
                    hs = slice(h * 512, (h + 1) * 512)
                    for k in range(KC):
                        nc.tensor.matmul(
                            cps, lhsT=aneg_p[:, k : k + 1], rhs=sq2g[k][:, hs],
                            start=(k == 0), stop=(k == KC - 1),
                        )
                    nc.scalar.activation(
                        ec2_row[:, g * 1024 + h * 512 : g * 1024 + (h + 1) * 512],
                        cps, AF.Exp,
                    )
            nc.vector.tensor_scalar_mul(ec2_row, ec2_row, var)
            nc.gpsimd.partition_broadcast(ec2b[:, ssl], ec2_row)

            last = s2 == NS - 1
            for m in range(MT):
                ot = None if last else outp.tile([128, 2048], f32, tag="ot",
                                                 bufs=3, name="ot")
                for g in range(2):
                    g0 = s2 * 2048 + g * 1024
                    ps = psum.tile([128, 1024], f32, tag="mainps")
                    for h in range(2):
                        sl = slice(g0 + h * 512, g0 + (h + 1) * 512)
                        pslice = ps[:, h * 512 : (h + 1) * 512]
                        for k in range(KC):
                            nc.tensor.matmul(
                                pslice,
                                lhsT=x1a_c[k][:, m * 128 : (m + 1) * 128],
                                rhs=x2_c[k][:, sl],
                                start=(k == 0), stop=(k == KC - 1),
                            )
                    if last:
                        # shorter per-tile drain chain for the final group
                        oth = outp.tile([128, 1024], f32, tag="oth", bufs=3,
                                        name="oth")
                        nc.scalar.activation(oth, ps, AF.Exp,
                                             bias=r1_t[:, m : m + 1])
                        nc.vector.tensor_mul(
                            oth, oth, ec2b[:, g0 : g0 + 1024])
                        nc.sync.dma_start(
                            out=out[m * 128 : (m + 1) * 128, g0 : g0 + 1024],
                            in_=oth)
                    else:
                        nc.scalar.activation(
                            ot[:, g * 1024 : (g + 1) * 1024], ps, AF.Exp,
                            bias=r1_t[:, m : m + 1],
                        )
                if not last:
                    nc.vector.tensor_mul(ot, ot, ec2b[:, ssl])
                    nc.sync.dma_start(
                        out=out[m * 128 : (m + 1) * 128, ssl], in_=ot)


_CACHE = {}


def _get_compiled():
    if "nc" in _CACHE:
        return _CACHE["nc"]
    import concourse.mybir as mybir
    import concourse.tile as tile
    from concourse import bacc

    f32 = mybir.dt.float32
    bf16 = mybir.dt.bfloat16
    nc = bacc.Bacc("TRN2", target_bir_lowering=False, debug=False,
                   enable_asserts=False)
    x1t = nc.dram_tensor("x1t", [DIM, ROWS], f32, kind="ExternalInput").ap()
    x2t = nc.dram_tensor("x2t", [DIM, M_COLS], bf16, kind="ExternalInput").ap()
    araw = nc.dram_tensor("alpha_raw", [DIM], f32, kind="ExternalInput").ap()
    vraw = nc.dram_tensor("variance_raw", [1], f32, kind="ExternalInput").ap()
    out = nc.dram_tensor("out", [ROWS, M_COLS], f32, kind="ExternalOutput").ap()

    with tile.TileContext(nc) as tc:
        build_ard_rbf(tc, out, x1t, x2t, araw, vraw, ROWS, M_COLS, DIM)
    nc.compile()
    _CACHE["nc"] = nc
    return nc


def kernel(x1, x2, alpha_raw, variance_raw):
    from concourse import bass_utils

    x1 = np.asarray(x1, dtype=np.float32)
    x2 = np.asarray(x2, dtype=np.float32)
    alpha_raw = np.ascontiguousarray(np.asarray(alpha_raw, dtype=np.float32))
    variance_raw = np.ascontiguousarray(
        np.asarray(variance_raw, dtype=np.float32))

    x1t_full = np.ascontiguousarray(x1.T)                      # [512, 8192] f32
    x2t_full = np.ascontiguousarray(x2.T).astype(ml_dtypes.bfloat16)

    nc = _get_compiled()
    in_maps = []
    for c in range(N_CORES):
        in_maps.append({
            "x1t": np.ascontiguousarray(x1t_full[:, c * ROWS : (c + 1) * ROWS]),
            "x2t": x2t_full,
            "alpha_raw": alpha_raw,
            "variance_raw": variance_raw,
        })
    res = bass_utils.run_bass_kernel_spmd(
        nc, in_maps, core_ids=list(range(N_CORES)),
        trace=bool(int(os.environ.get("ARD_TRACE", "0"))),
        tmpdir=os.environ.get("ARD_TMPDIR"),
    )
    _CACHE["last_results"] = res
    out = np.concatenate([res.results[c]["out"] for c in range(N_CORES)], axis=0)
    return out


if __name__ == "__main__":
    rng = np.random.default_rng(0)
    ins = {
        "x1": rng.standard_normal((N_ROWS, DIM), dtype=np.float32),
        "x2": rng.standard_normal((M_COLS, DIM), dtype=np.float32),
        "alpha_raw": rng.standard_normal((DIM,), dtype=np.float32),
        "variance_raw": rng.random((1,), dtype=np.float32),
    }
    o = kernel(**ins)
    print(o.shape, o.dtype)


# revision 15
# speedup vs baseline: 1.0293x; 1.0074x over previous
"""ARD-RBF kernel matrix on 8 Trainium2 NeuronCores.

out = variance * exp(-0.5 * (sq1[:,None] + sq2[None,:] - 2*cross))
    = exp(cross + (-0.5*sq1 + ln(var))[:,None] + (-0.5*sq2)[None,:])

with alpha = softmax(softplus(alpha_raw)), variance = variance_raw[0]**2,
cross = (x1*alpha) @ x2.T, sq1 = (x1*x1)@alpha, sq2 = (x2*x2)@alpha.

Device strategy (per core, rows of x1 sharded 8 ways):
  - host ships x1.T shard [512, 1024] f32 and x2.T [512, 8192] bf16
    (pure layout/precision prep; every FLOP of the reference runs on
    device).
  - alpha/softplus/softmax computed on-chip on a [1,512] row; scattered
    to [128,4] per-partition layout via a DRAM bounce.
  - r1 = -0.5*sq1 + ln(var) via M=1 matmuls (alpha as stationary) plus a
    K=2 hi/lo matmul carrying ln(var); applied as the per-partition f32
    bias of the final ScalarE Exp activation.
  - c2 = -0.5*sq2 via M=1 matmuls; split hi/lo into two bf16 rows that
    augment the main GEMM's contraction (exactly adds c2[j] per column).
  - main GEMM: for each output tile, 4 K-chunk bf16 matmuls + 1 K=2
    augment matmul accumulate in PSUM; one Exp activation PSUM->SBUF with
    bias; DMA to DRAM.
"""

import os
import sys

import numpy as np

sys.path.insert(0, "/opt/trn_rl_repo")

import ml_dtypes

N_CORES = 8
N_ROWS, M_COLS, DIM = 8192, 8192, 512
ROWS = N_ROWS // N_CORES  # 1024 rows of x1 per core


def build_ard_rbf(tc, out, x1t, x2t, araw, vraw, rows, m_cols, dim):
    """Emit the per-core kernel. APs: out [rows, m_cols] f32,
    x1t [dim, rows] f32, x2t [dim, m_cols] bf16, araw [dim] f32,
    vraw [1] f32.

    Uses the unnormalized-softmax trick: with u = 1+e^araw (softplus'd
    softmax numerator) and S = sum(u), every alpha-weighted sum equals
    (1/S) * (u-weighted sum). The GEMM runs u-weighted; 1/S rides the
    final Exp activation's per-partition f32 scale. This keeps the
    pre-matmul dependency chain to ~5 hops.
    """
    import concourse.mybir as mybir

    nc = tc.nc
    f32 = mybir.dt.float32
    bf16 = mybir.dt.bfloat16
    AF = mybir.ActivationFunctionType

    KC = dim // 128          # contraction chunks (4)
    MT = rows // 128         # output row tiles per core (8)
    NG = m_cols // 1024      # 1024-col groups (8)

    with (
        tc.tile_pool(name="const", bufs=1) as const,
        tc.tile_pool(name="x2pool", bufs=1) as x2pool,
        tc.tile_pool(name="work", bufs=3) as work,
        tc.tile_pool(name="outp", bufs=4) as outp,
        tc.tile_pool(name="psum", bufs=3, space="PSUM") as psum,
        tc.tile_pool(name="psmall", bufs=2, space="PSUM") as psmall,
    ):
        # ---------------- bulk input loads first (HWDGE via sync) ----------
        x2_c = [
            x2pool.tile([128, m_cols], bf16, tag=f"x2{k}", name=f"x2_{k}")
            for k in range(KC)
        ]
        for g in range(NG):
            gsl = slice(g * 1024, (g + 1) * 1024)
            for k in range(KC):
                nc.sync.dma_start(
                    out=x2_c[k][:, gsl],
                    in_=x2t[k * 128 : (k + 1) * 128, gsl],
                )
        x1t_c = []
        for k in range(KC):
            xt = const.tile([128, rows], f32, tag=f"x1t{k}", name=f"x1t_{k}")
            nc.sync.dma_start(out=xt, in_=x1t[k * 128 : (k + 1) * 128, :])
            x1t_c.append(xt)

        # ---------------- u = 1 + exp(araw); critical path ------------------
        # small dependent DMAs go on SWDGE (gpsimd): separate queues + sems,
        # so they are not FIFO'd behind the bulk loads on HWDGE channels.
        a_row = const.tile([1, dim], f32)
        nc.gpsimd.dma_start(out=a_row, in_=araw.rearrange("(a d) -> a d", a=1))
        e0 = const.tile([1, dim], f32)
        sm = const.tile([1, 1], f32)
        nc.scalar.activation(e0, a_row, AF.Exp, accum_out=sm)
        # scatter e0 [1, dim] -> [128, KC] with one small DMA per chunk
        e_p = const.tile([128, KC], f32)
        for k in range(KC):
            nc.gpsimd.dma_start(
                out=e_p[:, k : k + 1],
                in_=e0[:, k * 128 : (k + 1) * 128],
            )
        u_p = const.tile([128, KC], f32)
        nc.vector.tensor_scalar_add(u_p, e_p, 1.0)
        uneg_p = const.tile([128, KC], bf16)
        nc.vector.tensor_scalar_mul(uneg_p, u_p, -0.5)

        # x1u = u * x1 (bf16 stationary operands for the main GEMM)
        x1a_c = []
        for k in range(KC):
            xa = const.tile([128, rows], bf16, tag=f"x1a{k}", name=f"x1a_{k}")
            nc.vector.tensor_scalar_mul(xa, x1t_c[k], u_p[:, k : k + 1])
            x1a_c.append(xa)

        # ---------------- rs = 1/(dim + sum(e)); off critical path ----------
        smd = const.tile([1, 1], f32)
        nc.vector.tensor_scalar_add(smd, sm, float(dim))
        rs = const.tile([1, 1], f32)
        nc.vector.reciprocal(rs, smd)
        rs128 = const.tile([128, 1], f32)
        nc.gpsimd.partition_broadcast(rs128, rs)
        vr = const.tile([1, 1], f32)
        nc.gpsimd.dma_start(out=vr, in_=vraw.rearrange("(a d) -> a d", a=1))
        var = const.tile([1, 1], f32)
        nc.vector.tensor_mul(var, vr, vr)

        # ---------------- r1u = -0.5*sum(u*x1^2); scale by rs --------------
        r1_row = const.tile([1, rows], f32)
        for h in range(rows // 512):
            ps = psmall.tile([1, 512], f32, tag="c2ps")
            for k in range(KC):
                sq = work.tile([128, 512], bf16, tag="sqx1")
                src = x1t_c[k][:, h * 512 : (h + 1) * 512]
                nc.vector.tensor_mul(sq, src, src)
                nc.tensor.matmul(
                    ps, lhsT=uneg_p[:, k : k + 1], rhs=sq,
                    start=(k == 0), stop=(k == KC - 1),
                )
            nc.vector.tensor_copy(r1_row[:, h * 512 : (h + 1) * 512], ps)
        r1_raw = const.tile([128, MT], f32)
        for t in range(MT):
            nc.gpsimd.dma_start(
                out=r1_raw[:, t : t + 1],
                in_=r1_row[:, t * 128 : (t + 1) * 128],
            )
        r1_t = const.tile([128, MT], f32)
        nc.vector.tensor_scalar_mul(r1_t, r1_raw, rs128)

        # ---------------- main loop: 2048-col super-groups, m inner --------
        NS = m_cols // 2048
        for s2 in range(NS):
            ssl = slice(s2 * 2048, (s2 + 1) * 2048)
            ec2_row = work.tile([1, 2048], f32, tag="ec2row", bufs=2)
            for g in range(2):
                g0 = s2 * 2048 + g * 1024
                sq2g = []
                for k in range(KC):
                    sq2 = work.tile(
                        [128, 1024], bf16, tag=f"sqx2{k}", bufs=2,
                        name=f"sq2_{k}",
                    )
                    nc.vector.tensor_mul(
                        sq2, x2_c[k][:, g0 : g0 + 1024], x2_c[k][:, g0 : g0 + 1024]
                    )
                    sq2g.append(sq2)
                for h in range(2):
                    cps = psmall.tile([1, 512], f32, tag="c2ps")
                    hs = slice(h * 512, (h + 1) * 512)
                    for k in range(KC):
                        nc.tensor.matmul(
                            cps, lhsT=uneg_p[:, k : k + 1], rhs=sq2g[k][:, hs],
                            start=(k == 0), stop=(k == KC - 1),
                        )
                    # exp(rs * c2u) = exp(c2); variance folded in below
                    nc.scalar.activation(
                        ec2_row[:, g * 1024 + h * 512 : g * 1024 + (h + 1) * 512],
                        cps, AF.Exp, scale=rs,
                    )
            nc.vector.tensor_scalar_mul(ec2_row, ec2_row, var)
            ec2b = work.tile([128, 2048], f32, tag="ec2b", bufs=2)
            nc.gpsimd.partition_broadcast(ec2b, ec2_row)

            last = s2 == NS - 1
            for m in range(MT):
                ot = None if last else outp.tile([128, 2048], f32, tag="ot",
                                                 bufs=3, name="ot")
                for g in range(2):
                    g0 = s2 * 2048 + g * 1024
                    ps = psum.tile([128, 1024], f32, tag="mainps")
                    for h in range(2):
                        sl = slice(g0 + h * 512, g0 + (h + 1) * 512)
                        pslice = ps[:, h * 512 : (h + 1) * 512]
                        for k in range(KC):
                            nc.tensor.matmul(
                                pslice,
                                lhsT=x1a_c[k][:, m * 128 : (m + 1) * 128],
                                rhs=x2_c[k][:, sl],
                                start=(k == 0), stop=(k == KC - 1),
                            )
                    if last:
                        oth = outp.tile([128, 1024], f32, tag="oth", bufs=3,
                                        name="oth")
                        nc.scalar.activation(oth, ps, AF.Exp,
                                             bias=r1_t[:, m : m + 1],
                                             scale=rs128)
                        nc.vector.tensor_mul(
                            oth, oth, ec2b[:, g * 1024 : (g + 1) * 1024])
                        nc.sync.dma_start(
                            out=out[m * 128 : (m + 1) * 128, g0 : g0 + 1024],
                            in_=oth)
                    else:
                        nc.scalar.activation(
                            ot[:, g * 1024 : (g + 1) * 1024], ps, AF.Exp,
                            bias=r1_t[:, m : m + 1], scale=rs128,
                        )
                if not last:
                    nc.vector.tensor_mul(ot, ot, ec2b)
                    nc.sync.dma_start(
                        out=out[m * 128 : (m + 1) * 128, ssl], in_=ot)


_CACHE = {}


def _get_compiled():
    if "nc" in _CACHE:
        return _CACHE["nc"]
    import concourse.mybir as mybir
    import concourse.tile as tile
    from concourse import bacc

    f32 = mybir.dt.float32
    bf16 = mybir.dt.bfloat16
    nc = bacc.Bacc("TRN2", target_bir_lowering=False, debug=False,
                   enable_asserts=False)
    x1t = nc.dram_tensor("x1t", [DIM, ROWS], f32, kind="ExternalInput").ap()
    x2t = nc.dram_tensor("x2t", [DIM, M_COLS], bf16, kind="ExternalInput").ap()
    araw = nc.dram_tensor("alpha_raw", [DIM], f32, kind="ExternalInput").ap()
    vraw = nc.dram_tensor("variance_raw", [1], f32, kind="ExternalInput").ap()
    out = nc.dram_tensor("out", [ROWS, M_COLS], f32, kind="ExternalOutput").ap()

    with tile.TileContext(nc) as tc:
        build_ard_rbf(tc, out, x1t, x2t, araw, vraw, ROWS, M_COLS, DIM)
    nc.compile()
    _CACHE["nc"] = nc
    return nc


def kernel(x1, x2, alpha_raw, variance_raw):
    from concourse import bass_utils

    x1 = np.asarray(x1, dtype=np.float32)
    x2 = np.asarray(x2, dtype=np.float32)
    alpha_raw = np.ascontiguousarray(np.asarray(alpha_raw, dtype=np.float32))
    variance_raw = np.ascontiguousarray(
        np.asarray(variance_raw, dtype=np.float32))

    x1t_full = np.ascontiguousarray(x1.T)                      # [512, 8192] f32
    x2t_full = np.ascontiguousarray(x2.T).astype(ml_dtypes.bfloat16)

    nc = _get_compiled()
    in_maps = []
    for c in range(N_CORES):
        in_maps.append({
            "x1t": np.ascontiguousarray(x1t_full[:, c * ROWS : (c + 1) * ROWS]),
            "x2t": x2t_full,
            "alpha_raw": alpha_raw,
            "variance_raw": variance_raw,
        })
    res = bass_utils.run_bass_kernel_spmd(
        nc, in_maps, core_ids=list(range(N_CORES)),
        trace=bool(int(os.environ.get("ARD_TRACE", "0"))),
        tmpdir=os.environ.get("ARD_TMPDIR"),
    )
    _CACHE["last_results"] = res
    out = np.concatenate([res.results[c]["out"] for c in range(N_CORES)], axis=0)
    return out


if __name__ == "__main__":
    rng = np.random.default_rng(0)
    ins = {
        "x1": rng.standard_normal((N_ROWS, DIM), dtype=np.float32),
        "x2": rng.standard_normal((M_COLS, DIM), dtype=np.float32),
        "alpha_raw": rng.standard_normal((DIM,), dtype=np.float32),
        "variance_raw": rng.random((1,), dtype=np.float32),
    }
    o = kernel(**ins)
    print(o.shape, o.dtype)
